# revision 1
# baseline (speedup 1.0000x reference)
"""Self-contained Trainium2 Bass kernel for the LSS voxel-pooling problem
(nn_DSFusionv2_28819230556604).

kernel(**inputs) takes the FULL unsharded inputs (numpy) and returns the
FULL [B, C, NZ, NY, NX] float32 output.

Strategy (8 NeuronCores, data-parallel over batch x depth-chunks):
  core c handles batch b = c//4 and depth range d in [12*(c%4), 12*(c%4)+12),
  all 6 cameras -> 72 (n,d) "slices" per core.

  The camera geometry here makes voxel indices separable per slice:
  x,y cell indices depend only on (n,d,w); the z in-bounds mask depends only
  on (n,d,h).  The host computes the indices (mirroring the reference's
  float32 ops exactly) and bakes them into tiny mask/one-hot operands.

  Device pipeline per core (all affine HWDGE DMAs, PE-centric):
    stage A: block-column mask matmuls reduce over h (and z-mask) while x
             streams through the PE -> colsum [72 slices, 44 w, 80 ch] in PSUM
    spill:   cast to bf16, round-trip through DRAM to transpose w onto
             partitions
    stage B: per-slice-pair one-hot matmuls combine duplicate cells within a
             slice (all w hitting the same BEV cell) -> compact per-slice cell
             rows, exact f32 accumulation
  Host merges the compact rows (cross-slice / cross-core duplicates) into the
  BEV canvas with one vectorized scatter-add over <=18K pre-summed rows.
"""
import os
import numpy as np
import ml_dtypes

# ---- problem constants (hardcoded from the reference config) ----
B, N, D, FH, FW, C = 2, 6, 48, 16, 44, 80
OGH, OGW = 256, 704
D_MIN, D_MAX = 2.0, 58.0
NX, NY, NZ = 256, 256, 1
LOWER = np.array([-51.2, -51.2, -10.0], np.float32)
DX = np.array([0.4, 0.4, 20.0], np.float32)

NCORE = 8
DCHUNK = D // (NCORE // B)        # 12
NSLICE = N * DCHUNK               # 72
NGROUP = NSLICE // 8              # 9
MCELL = 64
NPAIR = NSLICE // 2               # 36
WC = FW * C                       # 3520
WL = 22
H1 = WL * C                       # 1760


def _frustum():
    ds = D_MIN + (D_MAX - D_MIN) / D * np.arange(D, dtype=np.float32)
    ds = np.broadcast_to(ds[:, None, None], (D, FH, FW))
    xs = np.broadcast_to(np.linspace(0, OGW - 1, FW, dtype=np.float32)[None, None, :], (D, FH, FW))
    ys = np.broadcast_to(np.linspace(0, OGH - 1, FH, dtype=np.float32)[None, :, None], (D, FH, FW))
    return np.stack([xs, ys, ds], -1)


def _geometry_indices(rots, trans, intrins, post_rots, post_trans):
    """Voxel indices, bit-matching the reference's float32 op sequence."""
    frustum = _frustum()
    pts = frustum[None, None] - post_trans[:, :, None, None, None, :]
    inv_post = np.linalg.inv(post_rots).astype(np.float32)
    pts = np.einsum('bnij,bndhwj->bndhwi', inv_post, pts).astype(np.float32)
    pts = np.concatenate([pts[..., :2] * pts[..., 2:3], pts[..., 2:3]], axis=-1)
    combine = np.einsum('bnij,bnjk->bnik', rots,
                        np.linalg.inv(intrins).astype(np.float32)).astype(np.float32)
    pts = np.einsum('bnij,bndhwj->bndhwi', combine, pts).astype(np.float32)
    geom = (pts + trans[:, :, None, None, None, :]).astype(np.float32)
    gi = ((geom - LOWER) / DX).astype(np.int32)
    kept = ((gi[..., 0] >= 0) & (gi[..., 0] < NX) &
            (gi[..., 1] >= 0) & (gi[..., 1] < NY) &
            (gi[..., 2] >= 0) & (gi[..., 2] < NZ))
    return gi, kept


def _build_core_plan(gi, kept, core):
    b = core // (NCORE // B)
    d0 = (core % (NCORE // B)) * DCHUNK
    zmask = np.zeros((NSLICE, FH), np.float32)
    cellxy = np.full((NSLICE, FW), -1, np.int64)
    for n in range(N):
        for dd in range(DCHUNK):
            d = d0 + dd
            s = n * DCHUNK + dd
            g = gi[b, n, d]
            k = kept[b, n, d]
            if not (g[..., 0] == g[0:1, :, 0]).all() or not (g[..., 1] == g[0:1, :, 1]).all():
                raise RuntimeError("structure violation: gi_x/gi_y vary with h")
            zok = (g[:, :, 2] >= 0) & (g[:, :, 2] < NZ)
            if not (zok == zok[:, 0:1]).all():
                raise RuntimeError("structure violation: z-ok varies with w")
            xyok = ((g[0, :, 0] >= 0) & (g[0, :, 0] < NX) &
                    (g[0, :, 1] >= 0) & (g[0, :, 1] < NY))
            if not (k == (zok[:, 0:1] & xyok[None, :])).all():
                raise RuntimeError("structure violation: kept not separable")
            zmask[s] = zok[:, 0].astype(np.float32)
            cellxy[s] = np.where(xyok, g[0, :, 1].astype(np.int64) * NX + g[0, :, 0], -1)

    Z = np.zeros((NGROUP, 128, NSLICE), np.float32)
    for g_ in range(NGROUP):
        for j in range(8):
            s = g_ * 8 + j
            Z[g_, j * FH:(j + 1) * FH, s] = zmask[s]

    O = np.zeros((128, NPAIR, 128), np.float32)
    out_cells = np.full((NSLICE, MCELL), -1, np.int64)
    for s in range(NSLICE):
        half, t = s % 2, s // 2
        ranks = {}
        for w in range(FW):
            c = cellxy[s, w]
            if c < 0:
                continue
            if c not in ranks:
                ranks[c] = len(ranks)
                out_cells[s, ranks[c]] = c
            O[64 * half + w, t, 64 * half + ranks[c]] = 1.0
    return dict(b=b, Z=Z, O=O, out_cells=out_cells)


def _build_nc():
    import concourse.bacc as bacc
    import concourse.mybir as mybir
    import concourse.tile as tile
    F32 = mybir.dt.float32
    BF16 = mybir.dt.bfloat16

    nc = bacc.Bacc(None, target_bir_lowering=True)
    x_d = nc.dram_tensor("x", [NGROUP * 128, WC], BF16, kind="ExternalInput")
    z_d = nc.dram_tensor("z", [128, NGROUP, NSLICE], BF16, kind="ExternalInput")
    o_d = nc.dram_tensor("o", [128, NPAIR, 128], BF16, kind="ExternalInput")
    tok_d = nc.dram_tensor("tokscratch", [NSLICE, WC], BF16)
    out_d = nc.dram_tensor("out", [128, NPAIR, C], F32, kind="ExternalOutput")

    with tile.TileContext(nc) as tc:
        with (
            tc.tile_pool(name="sbuf", bufs=1) as pool,
            tc.tile_pool(name="xin", bufs=6) as xpool,
            tc.tile_pool(name="psum", bufs=1, space="PSUM") as psum,
        ):
            ztile = pool.tile([128, NGROUP, NSLICE], BF16)
            nc.scalar.dma_start(ztile[:], z_d[:])
            otile = pool.tile([128, NPAIR, 128], BF16)
            nc.scalar.dma_start(otile[:], o_d[:])
            colT = pool.tile([128, NPAIR, C], BF16)
            nc.vector.memset(colT[:], 0.0)

            psumA = psum.tile([128, WC], F32, tag="ps")
            tokbf = pool.tile([NSLICE, WC], BF16)
            for g in range(NGROUP - 1):
                xg = xpool.tile([128, WC], BF16)
                nc.sync.dma_start(xg[:], x_d[128 * g:128 * (g + 1), :])
                for o in range(0, WC, 512):
                    w = min(512, WC - o)
                    nc.tensor.matmul(
                        psumA[0:NSLICE, o:o + w],
                        ztile[:, g, :], xg[:, o:o + w],
                        start=(g == 0), stop=False,
                        skip_group_check=True,
                    )
            g = NGROUP - 1
            xg = xpool.tile([128, WC], BF16)
            nc.sync.dma_start(xg[:], x_d[128 * g:128 * (g + 1), :])
            for o in range(0, WC, 512):
                w = min(512, WC - o)
                nc.tensor.matmul(
                    psumA[0:NSLICE, o:o + w],
                    ztile[:, g, :], xg[:, o:o + w],
                    start=False, stop=True,
                    skip_group_check=True,
                )
                if o + w == 2048:
                    nc.vector.tensor_copy(tokbf[:, 0:H1], psumA[0:NSLICE, 0:H1])
                    nc.sync.dma_start(tok_d[:, 0:H1], tokbf[:, 0:H1])
            nc.vector.tensor_copy(tokbf[:, H1:], psumA[0:NSLICE, H1:])
            nc.sync.dma_start(tok_d[:, H1:], tokbf[:, H1:])

            tok4 = tok_d[:].rearrange("(t two) (w c) -> two w t c", two=2, c=C)
            nc.sync.dma_start(colT[0:WL, :, :], tok4[0][0:WL])
            nc.scalar.dma_start(colT[64:64 + WL, :, :], tok4[1][0:WL])
            nc.sync.dma_start(colT[WL:FW, :, :], tok4[0][WL:FW])
            nc.scalar.dma_start(colT[64 + WL:64 + FW, :, :], tok4[1][WL:FW])

            outbuf = pool.tile([128, NPAIR, C], F32)
            for v in range(2):
                psumB = psum.tile([128, 3, 512], F32, tag="ps")
                for u in range(18):
                    t = 18 * v + u
                    nc.tensor.matmul(
                        psumB[:, u // 6, C * (u % 6):C * (u % 6) + C],
                        otile[:, t, :], colT[:, t, :],
                        start=True, stop=True, skip_group_check=True,
                    )
                nc.vector.tensor_copy(
                    outbuf[:, 18 * v:18 * (v + 1), :]
                    .rearrange("p (b t) c -> p b (t c)", b=3),
                    psumB[:, :, 0:6 * C],
                )
                nc.sync.dma_start(out_d[:, 18 * v:18 * (v + 1), :],
                                  outbuf[:, 18 * v:18 * (v + 1), :])
    nc.compile()
    return nc


_NC_CACHE = None
_LAST_EXEC_NS = None


def kernel(x, rots, trans, intrins, post_rots, post_trans):
    global _NC_CACHE, _LAST_EXEC_NS
    x = np.asarray(x)
    rots = np.asarray(rots, np.float32)
    trans = np.asarray(trans, np.float32)
    intrins = np.asarray(intrins, np.float32)
    post_rots = np.asarray(post_rots, np.float32)
    post_trans = np.asarray(post_trans, np.float32)

    gi, kept = _geometry_indices(rots, trans, intrins, post_rots, post_trans)
    plans = [_build_core_plan(gi, kept, c) for c in range(NCORE)]

    xb = x.astype(ml_dtypes.bfloat16)
    inmaps = []
    for core, plan in zip(range(NCORE), plans):
        b = core // (NCORE // B)
        d0 = (core % (NCORE // B)) * DCHUNK
        xc = np.ascontiguousarray(
            xb[b, :, d0:d0 + DCHUNK].reshape(NSLICE * FH, WC))
        inmaps.append({
            "x": xc,
            "z": np.ascontiguousarray(plan["Z"].transpose(1, 0, 2)).astype(ml_dtypes.bfloat16),
            "o": plan["O"].astype(ml_dtypes.bfloat16),
        })

    if _NC_CACHE is None:
        _NC_CACHE = _build_nc()
    from concourse.bass_utils import run_bass_kernel_spmd
    trace = bool(int(os.environ.get("LSS_TRACE", "0")))
    if not trace:
        # the NTFF trace path needs antenv.axon_hooks, absent in this image;
        # make sure a global BASS_TRACE=1 can't route us there
        os.environ["BASS_NEVER_TRACE"] = "1"
    res = run_bass_kernel_spmd(_NC_CACHE, inmaps, core_ids=list(range(NCORE)),
                               trace=trace)
    _LAST_EXEC_NS = res.exec_time_ns

    # host merge: compact per-slice cell rows -> BEV canvas
    canvas = np.zeros((B, NY * NX, C), np.float64)
    for r, plan in zip(res.results, plans):
        dev = np.asarray(r["out"])               # [128, NPAIR, C]
        rows = np.zeros((NSLICE, MCELL, C), np.float32)
        for s in range(NSLICE):
            rows[s] = dev[64 * (s % 2):64 * (s % 2) + MCELL, s // 2, :]
        oc = plan["out_cells"].reshape(-1)
        m = oc >= 0
        np.add.at(canvas[plan["b"]], oc[m], rows.reshape(-1, C)[m].astype(np.float64))
    out = (canvas.reshape(B, NY, NX, C).transpose(0, 3, 1, 2)[:, :, None]
           .astype(np.float32))
    return np.ascontiguousarray(out.reshape(B, C, NZ, NY, NX))

